# revision 24
# baseline (speedup 1.0000x reference)
"""Trainium2 Bass kernel for quantized-MoE Bottleneck (nn_Bottleneck_37503654429269).

v8 layout (mo-pair pipelined GN, biased-cast quantization):
- fp16 integer matmuls; quantized values stored as q+1024 so the fp16 RNE
  cast itself performs round-to-int (ulp=1 in [1024, 2048)).
- The +1024 bias is removed from every conv psum by an exact rank-2
  correction matmul (rs = a*512 + b split; fp16-exact rows vs rhs consts
  [-QB, -16QB]); BN affines see unbiased psums.
- Quant stages are 2 ops: ACT Relu-affine -> DVE (add 1024, min 1024+m)->fp16.
- GN stats/final processed per mo-pair (one GN channel-group): each pair's
  stats+final overlaps the next pair's conv3 matmuls on the PE.
- Small group's x is DMA'd and quantized first so conv1 starts early.
- gnb folded into the PQ outer-product matmul; Sqrt table prefetched.
- Final: DVE bf16 affine -> +x(bf16) -> relu; bf16 store, host upcasts.
"""

import numpy as np

BITS = (2, 4, 8)
EPS = 1e-5
B, C_IN, H, W = 32, 1024, 14, 14
WIDTH, OUTC = 256, 1024
PIX = H * W  # 196
NCORES = 8
QB = 1024.0  # quantization bias (fp16 ulp=1 in [1024, 2048))

_NC_CACHE = {}


# ----------------------------------------------------------------------------
# Device program
# ----------------------------------------------------------------------------

def _build_nc(group_sizes):
    from contextlib import ExitStack
    import concourse.bacc as bacc
    import concourse.mybir as mybir
    import concourse.tile as tile

    F32 = mybir.dt.float32
    F16 = mybir.dt.float16
    BF16 = mybir.dt.bfloat16
    ALU = mybir.AluOpType
    ACT = mybir.ActivationFunctionType

    NG = len(group_sizes)
    NS = sum(group_sizes)
    assert NS == 4
    slot0 = [sum(group_sizes[:g]) for g in range(NG)]
    groups = [list(range(slot0[g], slot0[g] + group_sizes[g])) for g in range(NG)]
    chunks = {g: [groups[g][i:i + 2] for i in range(0, len(groups[g]), 2)]
              for g in range(NG)}
    GORD = sorted(range(NG), key=lambda g: group_sizes[g])  # small group first

    nc = bacc.Bacc("TRN2", target_bir_lowering=False, debug=False,
                   num_devices=NCORES)

    # ---- dram tensors
    x_d = nc.dram_tensor("x", [128, 8, 4 * PIX], F32, kind="ExternalInput")
    xb_d = nc.dram_tensor("xb", [128, 8, 4 * PIX], BF16, kind="ExternalInput")
    wa_d = nc.dram_tensor("wa", [NG, 128, 8704], F16, kind="ExternalInput")
    rs_d = nc.dram_tensor("rs", [NG, 2, 1536], F16, kind="ExternalInput")
    crb_d = nc.dram_tensor("crb", [2, 2 * PIX], F16, kind="ExternalInput")
    # packed per-partition consts: xs[NG] xbm[NG] a1[2NG] b1[2NG] a2[2NG] b2[2NG]
    NCC = 2 * NG + 4 * (2 * NG)
    cc_d = nc.dram_tensor("cc", [128, NCC], F32, kind="ExternalInput")
    # row0: gng[1024] c3e[m,slot][16] c3e2[16]; row1: gnb[1024] zeros[32]
    gg_d = nc.dram_tensor("gg", [2, 1024 + 32], F32, kind="ExternalInput")
    out_d = nc.dram_tensor("out", [128, 8, 4 * PIX], BF16, kind="ExternalOutput")

    with tile.TileContext(nc) as tc, ExitStack() as ctx:
        res = ctx.enter_context(tc.tile_pool(name="res", bufs=1))
        rot = ctx.enter_context(tc.tile_pool(name="rot", bufs=3))
        ro2 = ctx.enter_context(tc.tile_pool(name="ro2", bufs=2))
        mmp = ctx.enter_context(tc.tile_pool(name="mmp", bufs=6, space="PSUM"))
        smp = ctx.enter_context(tc.tile_pool(name="smp", bufs=1, space="PSUM"))

        # ---- loads (order matters for schedule priority)
        CC = res.tile([128, NCC], F32, name="CC", tag="CC")
        nc.sync.dma_start(out=CC, in_=cc_d.ap())
        o = 0
        XS = CC[:, o:o + NG]; o += NG
        XBM = CC[:, o:o + NG]; o += NG
        A1 = CC[:, o:o + 2 * NG].rearrange("p (m g) -> p m g", m=2); o += 2 * NG
        B1 = CC[:, o:o + 2 * NG].rearrange("p (m g) -> p m g", m=2); o += 2 * NG
        A2 = CC[:, o:o + 2 * NG].rearrange("p (m g) -> p m g", m=2); o += 2 * NG
        B2 = CC[:, o:o + 2 * NG].rearrange("p (m g) -> p m g", m=2); o += 2 * NG

        GG = res.tile([2, 1024 + 32], F32, name="GG", tag="GG")
        nc.sync.dma_start(out=GG, in_=gg_d.ap())

        WA = [res.tile([128, 8704], F16, name=f"WA_{g}", tag=f"WA_{g}")
              for g in range(NG)]
        W1 = [WA[g][:, 0:2048].rearrange("p (k c) -> p k c", k=8)
              for g in range(NG)]
        W2 = [WA[g][:, 2048:6656].rearrange("p (t k c) -> p t k c", t=9, k=2)
              for g in range(NG)]
        W3 = [WA[g][:, 6656:8704].rearrange("p (k c) -> p k c", k=2)
              for g in range(NG)]
        RSC = [res.tile([2, 1536], F16, name=f"RSC_{g}", tag=f"RSC_{g}")
               for g in range(NG)]
        XG = [res.tile([128, 8, group_sizes[g] * PIX], F32, name=f"XG{g}",
                       tag=f"XG{g}") for g in range(NG)]
        XBh = [res.tile([128, 2, 4 * PIX], BF16, name=f"XB{h}", tag=f"XB{h}")
               for h in range(4)]

        def XBv(mo):
            return XBh[mo // 2][:, mo % 2, :]

        CRB = res.tile([2, 2 * PIX], F16, name="CRB", tag="CRB")
        # x on the sync queue, smallest group first; weights/consts on the
        # (otherwise idle) gpsimd queue so descriptor issue overlaps
        for g in GORD:
            nc.gpsimd.dma_start(out=WA[g], in_=wa_d.ap()[g])
            nc.gpsimd.dma_start(out=RSC[g], in_=rs_d.ap()[g])
        nc.gpsimd.dma_start(out=CRB, in_=crb_d.ap())
        for g in GORD:
            ns = group_sizes[g]
            for h in range(4):
                nc.sync.dma_start(
                    out=XG[g][:, 2 * h:2 * h + 2, :],
                    in_=x_d.ap()[:, 2 * h:2 * h + 2,
                                 slot0[g] * PIX:(slot0[g] + ns) * PIX])
        for h in range(4):
            nc.sync.dma_start(out=XBh[h],
                              in_=xb_d.ap()[:, 2 * h:2 * h + 2, :])
        ONES = res.tile([128, 1], F32, name="ONES", tag="ONES")
        nc.vector.memset(ONES, 1.0)
        # prefetch the sqrt table set so the real Sqrt doesn't stall
        WARM = res.tile([1, 1], F32, name="WARM", tag="WARM")
        nc.scalar.activation(out=WARM, in_=ONES[0:1, 0:1], func=ACT.Sqrt,
                             bias=0.0, scale=1.0)

        # ---------------- x quantization ----------------
        # xq = clamp(round(relu(x)*(lv-1)), 0, lv-1) + QB  in fp16
        Xqh = [[None] * NG for _ in range(4)]
        for h in range(4):
            for g in range(NG):
                ns = group_sizes[g]
                Xqh[h][g] = res.tile([128, 2, ns * PIX], F16,
                                     name=f"Xq{h}_{g}", tag=f"Xq{h}_{g}")
        for g in GORD:
            ns = group_sizes[g]
            for h in range(4):
                u = rot.tile([128, 2, ns * PIX], F32, name="xu", tag=f"xu{g}")
                nc.scalar.activation(out=u, in_=XG[g][:, 2 * h:2 * h + 2, :],
                                     func=ACT.Relu,
                                     bias=0.0, scale=XS[:, g:g + 1])
                nc.vector.tensor_scalar(out=Xqh[h][g], in0=u, scalar1=QB,
                                        scalar2=XBM[:, g:g + 1],
                                        op0=ALU.add, op1=ALU.min)

        def Xq(kt, g):
            return Xqh[kt // 2][g][:, kt % 2, :]

        # ---------------- conv1 + bn1 + quant ----------------
        HP = [[None] * NG for _ in range(2)]
        for mo in range(2):
            for g in range(NG):
                ns = group_sizes[g]
                hp = res.tile([128, ns, 16, 18], F16, name=f"HP{mo}_{g}",
                              tag=f"HP{mo}_{g}")
                nc.vector.memset(hp, QB)
                HP[mo][g] = hp

        def bn_relu(ps_flat, g, mo, A, Bt, nchunk):
            tpr = rot.tile([128, nchunk * PIX], F32, name="tpr", tag="tpr")
            nc.scalar.activation(out=tpr, in_=ps_flat, func=ACT.Relu,
                                 bias=Bt[:, mo, g:g + 1], scale=A[:, mo, g:g + 1])
            return tpr

        for g in GORD:
            for mo in range(2):
                for ch in chunks[g]:
                    nchunk = len(ch)
                    c0 = ch[0] - slot0[g]
                    ps = mmp.tile([128, nchunk * PIX], F32, name="c1ps",
                                  tag="mm")
                    for kt in range(8):
                        nc.tensor.matmul(
                            ps,
                            W1[g][:, kt, mo * 128:(mo + 1) * 128],
                            Xq(kt, g)[:, c0 * PIX:(c0 + nchunk) * PIX],
                            start=(kt == 0), stop=False)
                    nc.tensor.matmul(
                        ps, RSC[g][:, mo * 128:(mo + 1) * 128],
                        CRB[:, 0:nchunk * PIX], start=False, stop=True)
                    tpr = bn_relu(ps, g, mo, A1, B1, nchunk)
                    nc.vector.tensor_scalar(
                        out=HP[mo][g][:, c0:c0 + nchunk, 1:15, 2:16],
                        in0=tpr.rearrange("p (s y x) -> p s y x", s=nchunk, y=14),
                        scalar1=QB, scalar2=XBM[:, g:g + 1],
                        op0=ALU.add, op1=ALU.min)

        # ---------------- conv2 + bn2 + quant ----------------
        Q2 = [[None] * NG for _ in range(2)]
        for mo in range(2):
            for g in range(NG):
                ns = group_sizes[g]
                Q2[mo][g] = res.tile([128, ns * PIX], F16, name=f"Q2{mo}_{g}",
                                     tag=f"Q2{mo}_{g}")
        for g in GORD:
            for mo in range(2):
                for ch in chunks[g]:
                    nchunk = len(ch)
                    c0 = ch[0] - slot0[g]
                    ps = mmp.tile([128, nchunk, 14, 14], F32, name="c2ps",
                                  tag="mm")
                    first = True
                    for ti, (dy, dx) in enumerate(
                            (dy, dx) for dy in range(3) for dx in range(3)):
                        for kt in range(2):
                            nc.tensor.matmul(
                                ps,
                                W2[g][:, ti, kt, mo * 128:(mo + 1) * 128],
                                HP[kt][g][:, c0:c0 + nchunk,
                                          dy:dy + 14, dx + 1:dx + 15],
                                start=first, stop=False)
                            first = False
                    nc.tensor.matmul(
                        ps.rearrange("p s y x -> p (s y x)"),
                        RSC[g][:, 256 + mo * 128:256 + (mo + 1) * 128],
                        CRB[:, 0:nchunk * PIX], start=False, stop=True)
                    tpr = bn_relu(ps.rearrange("p s y x -> p (s y x)"),
                                  g, mo, A2, B2, nchunk)
                    nc.vector.tensor_scalar(
                        out=Q2[mo][g][:, c0 * PIX:(c0 + nchunk) * PIX],
                        in0=tpr, scalar1=QB, scalar2=XBM[:, g:g + 1],
                        op0=ALU.add, op1=ALU.min)

        # ---------------- conv3 + GN, pipelined by mo-pair ----------------
        S3 = [[None] * NG for _ in range(8)]
        for g in range(NG):
            ns = group_sizes[g]
            for mo in range(8):
                S3[mo][g] = res.tile([128, ns * PIX], BF16, name=f"S3_{mo}_{g}",
                                     tag=f"S3_{mo}_{g}")
        # per pair mp: cols [mp*16 + o*4 + slot] sums, [mp*16 + 8 + o*4 + slot] sq
        STT = res.tile([128, 64], F32, name="STT", tag="STT")
        OT = res.tile([128, 8, 4 * PIX], BF16, name="OT", tag="OT")

        def emit_conv3(mp):
            for mo in (2 * mp, 2 * mp + 1):
                for g in GORD:
                    for ch in chunks[g]:
                        nchunk = len(ch)
                        c0 = ch[0] - slot0[g]
                        ps = mmp.tile([128, nchunk * PIX], F32, name="c3ps",
                                      tag="mm")
                        for kt in range(2):
                            nc.tensor.matmul(
                                ps,
                                W3[g][:, kt, mo * 128:(mo + 1) * 128],
                                Q2[kt][g][:, c0 * PIX:(c0 + nchunk) * PIX],
                                start=(kt == 0), stop=False)
                        nc.tensor.matmul(
                            ps,
                            RSC[g][:, 512 + mo * 128:512 + (mo + 1) * 128],
                            CRB[:, 0:nchunk * PIX],
                            start=False, stop=True)
                        for ci in range(nchunk):
                            si = c0 + ci
                            slot = slot0[g] + si
                            pslice = ps[:, ci * PIX:(ci + 1) * PIX]
                            sslice = S3[mo][g][:, si * PIX:(si + 1) * PIX]
                            col = mp * 16 + (mo % 2) * 4 + slot
                            acc = STT[:, col:col + 1]
                            if mo % 2 == 0:
                                nc.scalar.activation(out=sslice, in_=pslice,
                                                     func=ACT.Identity,
                                                     bias=0.0, scale=1.0,
                                                     accum_out=acc)
                            else:
                                nc.vector.tensor_scalar(out=sslice, in0=pslice,
                                                        scalar1=1.0, scalar2=0.0,
                                                        op0=ALU.mult, op1=ALU.add,
                                                        accum_out=acc)
                            jk = rot.tile([128, PIX], BF16, name="jk", tag="jk")
                            nc.vector.scalar_tensor_tensor(
                                out=jk, in0=sslice, scalar=1.0, in1=sslice,
                                op0=ALU.mult, op1=ALU.mult,
                                accum_out=STT[:, col + 8:col + 9])

        def emit_stats_final(mp):
            # ---------- stats for this pair's GN channel-group ----------
            red = smp.tile([1, 16], F32, name="red", tag="red")
            nc.tensor.matmul(red, ONES, STT[:, mp * 16:(mp + 1) * 16],
                             start=True, stop=True)
            Tg = res.tile([1, 16], F32, name=f"Tg{mp}", tag=f"Tg{mp}")
            nc.scalar.activation(out=Tg, in_=red, func=ACT.Identity,
                                 bias=0.0, scale=1.0)
            # pair-add parity: [1, k(2), o(2), slot(4)] -> TBS [1, k, slot]
            TBS = res.tile([1, 8], F32, name=f"TBS{mp}", tag=f"TBS{mp}")
            tgv = Tg.rearrange("p (k o s) -> p k o s", k=2, o=2)
            nc.vector.tensor_tensor(
                out=TBS.rearrange("p (k s) -> p k s", k=2),
                in0=tgv[:, :, 0, :], in1=tgv[:, :, 1, :], op=ALU.add)
            # mean = S/50176 ; e2 = SQ/50176 ; var = e2 - mean^2
            SC2 = res.tile([1, 8], F32, name=f"SC2{mp}", tag=f"SC2{mp}")
            nc.vector.tensor_scalar(out=SC2, in0=TBS,
                                    scalar1=1.0 / (2 * 128 * PIX),
                                    scalar2=None, op0=ALU.mult)
            MEAN = SC2[:, 0:4]
            E2 = SC2[:, 4:8]
            VAR = ro2.tile([1, 4], F32, name="VAR", tag="VAR")
            nc.vector.tensor_tensor(out=VAR, in0=MEAN, in1=MEAN, op=ALU.mult)
            nc.vector.tensor_tensor(out=VAR, in0=E2, in1=VAR, op=ALU.subtract)
            # rc = 1/sqrt(var*c3e^2 + eps)
            nc.vector.tensor_tensor(
                out=VAR, in0=VAR,
                in1=GG[0:1, 1024 + 16 + mp * 4:1024 + 16 + (mp + 1) * 4],
                op=ALU.mult)
            nc.vector.tensor_scalar(out=VAR, in0=VAR, scalar1=EPS,
                                    scalar2=None, op0=ALU.add)
            SD = ro2.tile([1, 4], F32, name="SD", tag="SD")
            nc.scalar.activation(out=SD, in_=VAR, func=ACT.Sqrt,
                                 bias=0.0, scale=1.0)
            RC = ro2.tile([1, 4], F32, name="RC", tag="RC")
            nc.vector.reciprocal(out=RC, in_=SD)
            # FVK: p0 = [fv_scale(4), fv_bias(4)] ; p1 = [0x4, 1x4]
            FVK = res.tile([2, 8], F32, name=f"FVK{mp}", tag=f"FVK{mp}")
            nc.vector.memset(FVK[:, 0:4], 0.0)
            nc.vector.memset(FVK[:, 4:8], 1.0)
            Fv = FVK[0:1, :]
            nc.vector.tensor_tensor(
                out=Fv[:, 0:4], in0=RC,
                in1=GG[0:1, 1024 + mp * 4:1024 + (mp + 1) * 4], op=ALU.mult)
            nc.vector.scalar_tensor_tensor(
                out=Fv[:, 4:8], in0=MEAN, scalar=-1.0,
                in1=Fv[:, 0:4], op0=ALU.mult, op1=ALU.mult)
            # P,Q outer products for the two mo of this pair (+gnb row)
            pqp = smp.tile([128, 2, 2, 4], F32, name="pqp", tag="pqp")
            fvv = FVK.rearrange("p (k s) -> p k s", k=2)
            for o in range(2):
                mo = 2 * mp + o
                nc.tensor.matmul(
                    pqp[:, o, :, :],
                    GG[:, mo * 128:(mo + 1) * 128],
                    fvv,
                    start=(o == 0), stop=(o == 1), skip_group_check=True)
            PQ = res.tile([128, 2, 2, 4], F32, name=f"PQ{mp}", tag=f"PQ{mp}")
            nc.scalar.activation(out=PQ, in_=pqp, func=ACT.Identity,
                                 bias=0.0, scale=1.0)

            # ---------- final affine + residual + relu + store ----------
            for o in range(2):
                mo = 2 * mp + o
                V = ro2.tile([128, 4 * PIX], BF16, name="V", tag="V")
                for g in range(NG):
                    for si in range(group_sizes[g]):
                        slot = slot0[g] + si
                        nc.vector.tensor_scalar(
                            out=V[:, slot * PIX:(slot + 1) * PIX],
                            in0=S3[mo][g][:, si * PIX:(si + 1) * PIX],
                            scalar1=PQ[:, o, 0, slot:slot + 1],
                            scalar2=PQ[:, o, 1, slot:slot + 1],
                            op0=ALU.mult, op1=ALU.add)
                VX = ro2.tile([128, 4 * PIX], BF16, name="VX", tag="VX")
                nc.vector.tensor_tensor(out=VX, in0=V, in1=XBv(mo),
                                        op=ALU.add)
                nc.vector.tensor_scalar(out=OT[:, mo, :], in0=VX,
                                        scalar1=0.0, scalar2=None,
                                        op0=ALU.max)
                nc.sync.dma_start(out=out_d.ap()[:, mo, :],
                                  in_=OT[:, mo, :])

        emit_conv3(0)
        emit_conv3(1)
        emit_stats_final(0)
        emit_conv3(2)
        emit_stats_final(1)
        emit_conv3(3)
        emit_stats_final(2)
        emit_stats_final(3)

    nc.compile()
    return nc


# ----------------------------------------------------------------------------
# Host side
# ----------------------------------------------------------------------------

def _quant_w(w, lv):
    n = max(lv // 2 - 1, 1)
    s = np.float32(np.abs(w).max()) + np.float32(1e-12)
    k = np.round((w.astype(np.float32) / s) * np.float32(n)).astype(np.float32)
    return k, np.float32(s) / np.float32(n)


def _assign_groups(mask):
    mask = np.asarray(mask).astype(np.int64)
    ids = {e: [int(i) for i in np.nonzero(mask == e)[0]] for e in range(3)}
    counts = [len(ids[e]) for e in range(3)]
    if all(c % 2 == 0 for c in counts):
        group_sizes = (2, 2)
        chunks2 = []
        for e in range(3):
            for j in range(0, counts[e], 2):
                chunks2.append((e, ids[e][j:j + 2]))
        assert len(chunks2) == 16
        core_samples = []
        core_experts = []
        for c in range(8):
            (ea, sa), (eb, sb) = chunks2[2 * c], chunks2[2 * c + 1]
            core_samples.append(sa + sb)
            core_experts.append([ea, eb])
        return group_sizes, core_samples, core_experts

    base = [c % 3 for c in counts]
    need = (8 - sum(base)) // 3
    t = [0, 0, 0]
    for e in range(3):
        cap = (counts[e] - base[e]) // 3
        take = min(cap, need)
        t[e] = take
        need -= take
        if need == 0:
            break
    assert need == 0
    b = [base[e] + 3 * t[e] for e in range(3)]
    a = [(counts[e] - b[e]) // 3 for e in range(3)]
    assert sum(a) == 8 and sum(b) == 8
    trip = []
    single = []
    for e in range(3):
        pos = 0
        for _ in range(a[e]):
            trip.append((e, ids[e][pos:pos + 3]))
            pos += 3
        for _ in range(b[e]):
            single.append((e, [ids[e][pos]]))
            pos += 1
        assert pos == counts[e]
    core_samples = []
    core_experts = []
    for c in range(8):
        ea, sa = trip[c]
        eb, sb = single[c]
        core_samples.append(sa + sb)
        core_experts.append([ea, eb])
    return (3, 1), core_samples, core_experts


def kernel(x, mask, w1, w2, w3, bn1_g, bn1_b, bn1_m, bn1_v,
           bn2_g, bn2_b, bn2_m, bn2_v, gn_g, gn_b):
    import ml_dtypes
    from concourse.bass_utils import run_bass_kernel_spmd

    bf16 = ml_dtypes.bfloat16
    f16 = np.float16
    f32 = np.float32
    f64 = np.float64
    x = np.asarray(x, f32)
    mask = np.asarray(mask)
    w1 = np.asarray(w1, f32)
    w2 = np.asarray(w2, f32)
    w3 = np.asarray(w3, f32)
    bn1 = [np.asarray(v, f32) for v in (bn1_g, bn1_b, bn1_m, bn1_v)]
    bn2 = [np.asarray(v, f32) for v in (bn2_g, bn2_b, bn2_m, bn2_v)]
    gn_g = np.asarray(gn_g, f32)
    gn_b = np.asarray(gn_b, f32)

    group_sizes, core_samples, core_experts = _assign_groups(mask)
    NG = len(group_sizes)

    lv_of = [2 ** b for b in BITS]
    K1, K2, K3 = {}, {}, {}
    CW = {}
    RS = {}
    for e in set(int(v) for v in np.asarray(mask)):
        lv = lv_of[e]
        k1, c1 = _quant_w(w1, lv)
        k2, c2 = _quant_w(w2, lv)
        k3, c3 = _quant_w(w3, lv)
        K1[e] = k1.reshape(256, 1024)
        K2[e] = k2.reshape(256, 256, 3, 3)
        K3[e] = k3.reshape(1024, 256)
        CW[e] = (c1, c2, c3)
        RS[e] = (K1[e].sum(axis=1, dtype=f64),
                 K2[e].sum(axis=(1, 2, 3), dtype=f64),
                 K3[e].sum(axis=1, dtype=f64))

    inv1 = bn1[0] / np.sqrt(bn1[3] + f32(EPS))
    bb1 = bn1[1] - bn1[2] * inv1
    inv2 = bn2[0] / np.sqrt(bn2[3] + f32(EPS))
    bb2 = bn2[1] - bn2[2] * inv2

    def pack_w(e):
        k1t = K1[e].T.reshape(8, 128, 256).transpose(1, 0, 2)
        k2t = K2[e].transpose(2, 3, 1, 0).reshape(9, 2, 128, 256)
        k2t = k2t.transpose(2, 0, 1, 3)
        k3t = K3[e].T.reshape(2, 128, 1024).transpose(1, 0, 2)

        # exact fp16 split rs = a*512 + b -> rows [s*b, s*a*32] vs
        # rhs consts [-QB, -16*QB]
        def rs_split(rs):
            sgn = np.sign(rs)
            ab = np.abs(rs)
            a_ = np.floor(ab / 512.0)
            b_ = ab - a_ * 512.0
            return np.stack([sgn * b_, sgn * a_ * 32.0])
        rsp = np.zeros((2, 1536))
        rsp[:, 0:256] = rs_split(RS[e][0])
        rsp[:, 256:512] = rs_split(RS[e][1])
        rsp[:, 512:1536] = rs_split(RS[e][2])
        wa = np.concatenate([
            np.ascontiguousarray(k1t).reshape(128, 2048),
            np.ascontiguousarray(k2t).reshape(128, 4608),
            np.ascontiguousarray(k3t).reshape(128, 2048)], axis=1)
        return (np.ascontiguousarray(wa).astype(f16), rsp.astype(f16))

    packed = {e: pack_w(e) for e in set(int(v) for v in np.asarray(mask))}

    in_maps = []
    for c in range(8):
        sids = core_samples[c]
        experts = core_experts[c]

        # x: [128, 8, 784]: (p, kt, slot*196+pix)
        xc = x[sids].reshape(4, 8, 128, PIX).transpose(2, 1, 0, 3) \
                    .reshape(128, 8, 4 * PIX).copy()

        wac = np.stack([packed[experts[g]][0] for g in range(NG)])
        rsc = np.stack([packed[experts[g]][1] for g in range(NG)])

        glv = [lv_of[experts[g]] for g in range(NG)]
        cc = np.zeros((128, 2 * NG + 8 * NG), f32)
        cc[:, 0:NG] = [lv - 1 for lv in glv]              # xs
        cc[:, NG:2 * NG] = [QB + (lv - 1) for lv in glv]  # xbm = QB + m
        a1 = np.zeros((128, 2, NG), f32)
        b1 = np.zeros((128, 2, NG), f32)
        a2 = np.zeros((128, 2, NG), f32)
        b2 = np.zeros((128, 2, NG), f32)
        for g in range(NG):
            e = experts[g]
            m = f64(glv[g] - 1)
            a1v = (inv1 * CW[e][0]).astype(f64)
            a2v = (inv2 * CW[e][1]).astype(f64)
            b1v = bb1.astype(f64) * m
            b2v = bb2.astype(f64) * m
            a1[:, :, g] = a1v.astype(f32).reshape(2, 128).T
            b1[:, :, g] = b1v.astype(f32).reshape(2, 128).T
            a2[:, :, g] = a2v.astype(f32).reshape(2, 128).T
            b2[:, :, g] = b2v.astype(f32).reshape(2, 128).T
        o = 2 * NG
        cc[:, o:o + 2 * NG] = a1.reshape(128, 2 * NG); o += 2 * NG
        cc[:, o:o + 2 * NG] = b1.reshape(128, 2 * NG); o += 2 * NG
        cc[:, o:o + 2 * NG] = a2.reshape(128, 2 * NG); o += 2 * NG
        cc[:, o:o + 2 * NG] = b2.reshape(128, 2 * NG); o += 2 * NG

        gg = np.zeros((2, 1024 + 32), f32)
        gg[0, 0:1024] = gn_g
        gg[1, 0:1024] = gn_b
        c3e_slot = np.zeros(4, f32)
        for g in range(NG):
            for si in range(group_sizes[g]):
                slot = sum(group_sizes[:g]) + si
                c3e_slot[slot] = CW[experts[g]][2] / f32(glv[g] - 1)
        gg[0, 1024:1024 + 16] = np.tile(c3e_slot, 4)
        gg[0, 1024 + 16:1024 + 32] = np.tile(c3e_slot * c3e_slot, 4)

        crb = np.zeros((2, 2 * PIX))
        crb[0, :] = -QB
        crb[1, :] = -16.0 * QB
        in_maps.append({
            "x": xc, "xb": xc.astype(bf16), "wa": wac,
            "rs": rsc, "crb": crb.astype(f16), "cc": cc, "gg": gg,
        })

    key = group_sizes
    if key not in _NC_CACHE:
        _NC_CACHE[key] = _build_nc(group_sizes)
    nc = _NC_CACHE[key]

    res = run_bass_kernel_spmd(nc, in_maps, core_ids=list(range(NCORES)))

    out = np.zeros((B, OUTC, H, W), f32)
    for c in range(8):
        oc = res.results[c]["out"].astype(f32)  # [128, 8, 4*PIX]
        oc = oc.reshape(128, 8, 4, PIX).transpose(2, 1, 0, 3) \
               .reshape(4, OUTC, H, W)
        for t, sid in enumerate(core_samples[c]):
            out[sid] = oc[t]
    return out
